# revision 12
# baseline (speedup 1.0000x reference)
# LSTM (BaseRNN over LSTMLR cell) Trainium2 Bass kernel.
#
# Problem: x[16, 4096, 128], W[128, 512], U[128, 512], biases[128] ->
#          (hs, cs) each [16, 4096, 128] (batch_first), h0 = c0 = 0.
#
# Strategy: data-parallel over batch, 2 sequences per NeuronCore on 8 cores.
# Per core:
#   Phase 1: xwb[g] = x @ W_g + b_g  precomputed for all timesteps with the
#            tensor engine (layout [H=128 partitions, (g, bb, t)], bf16).
#   Phase 2: the sequential recurrence, one step at a time:
#            z = xwb[t] + h_{t-1} @ U   (PSUM accumulation: an identity-
#                matmul injects xwb into the bank, 4 gate matmuls accumulate)
#            all four gates through ONE sigmoid (tanh folded via
#            tanh(z) = 1 - 2*sigmoid(-2z); U_g/W_g/b_g pre-scaled by -2)
#            c = f*c + i - 2*(i*m)      (DVE)
#            h = o * tanh(c)            (ACT + DVE)
# Everything (xwb, hs, cs) stays resident in SBUF; outputs DMA out at the end.
#
# Constraint honored throughout: walrus rejects matmuls carrying more than
# one semaphore wait (self-loading InstMatmult puts waits on the LDWEIGHTS
# struct).  So every matmul operand is produced by a single engine (ACT),
# except the recurrent hT (DVE) whose wait rides on the gate matmuls after
# the identity matmul has already absorbed the ACT wait.

import numpy as np
import ml_dtypes
from contextlib import ExitStack

B, T, D, H = 16, 4096, 128, 128
NCORES = 8
BPC = B // NCORES  # sequences per core
G4 = 4 * H

UNROLL = 32          # recurrence steps per hardware-loop iteration
USE_FOR_I = True     # hardware loop vs full python unroll

_CACHED = {}


def _build_program():
    import concourse.bass as bass
    import concourse.tile as tile
    from concourse import bacc, mybir

    f32 = mybir.dt.float32
    bf16 = mybir.dt.bfloat16
    AF = mybir.ActivationFunctionType
    OP = mybir.AluOpType

    nc = bacc.Bacc(
        "TRN2",
        target_bir_lowering=False,
        debug=False,
        num_devices=NCORES,
    )

    xt_d = nc.dram_tensor("xt", [D, BPC, T], f32, kind="ExternalInput")
    wb_d = nc.dram_tensor("wb", [D, G4], bf16, kind="ExternalInput")
    ub_d = nc.dram_tensor("ub", [H, G4], bf16, kind="ExternalInput")
    bias_d = nc.dram_tensor("bias", [H, 4], f32, kind="ExternalInput")
    idb_d = nc.dram_tensor("idb", [128, 128], bf16, kind="ExternalInput")
    hs_d = nc.dram_tensor("hs", [H, BPC, T], f32, kind="ExternalOutput")
    cs_d = nc.dram_tensor("cs", [H, BPC, T], f32, kind="ExternalOutput")

    with tile.TileContext(nc) as tc, ExitStack() as ctx:
        const = ctx.enter_context(tc.tile_pool(name="const", bufs=1))
        big = ctx.enter_context(tc.tile_pool(name="big", bufs=1))

        # DMA constants into *_raw, then ACT-copy so that every matmul
        # operand has ACT as its sole producer (single-wait rule).
        ub_raw = const.tile([128, G4], bf16, tag="ub_raw")
        wb_raw = const.tile([128, G4], bf16, tag="wb_raw")
        idb_raw = const.tile([128, 128], bf16, tag="idb_raw")
        bias_sb = const.tile([128, 4], f32, tag="bias")
        nc.sync.dma_start(ub_raw[:], ub_d.ap())
        nc.sync.dma_start(wb_raw[:], wb_d.ap())
        nc.sync.dma_start(idb_raw[:], idb_d.ap())
        nc.sync.dma_start(bias_sb[:], bias_d.ap())

        ub_sb = const.tile([128, G4], bf16, tag="ub")
        wb_sb = const.tile([128, G4], bf16, tag="wb")
        idb_sb = const.tile([128, 128], bf16, tag="idb")
        nc.scalar.activation(ub_sb[:], ub_raw[:], AF.Copy)
        nc.scalar.activation(wb_sb[:], wb_raw[:], AF.Copy)
        nc.scalar.activation(idb_sb[:], idb_raw[:], AF.Copy)

        # x-projection buffer, [128, g, bb, t], bf16, biases folded in.
        xwb = big.tile([128, 4, BPC, T], bf16, tag="xwb")
        # Outputs kept fully resident in SBUF until the end.
        hsb = big.tile([128, BPC, T], f32, tag="hsb")
        csb = big.tile([128, BPC, T], f32, tag="csb")

        # ---------------- Phase 1: xwb[g] = x @ W_g + b_g ----------------
        with tc.tile_pool(name="p1in", bufs=3) as p1in, \
             tc.tile_pool(name="p1xt", bufs=3) as p1xt, \
             tc.tile_pool(name="p1ps", bufs=4, space="PSUM") as p1ps:
            xt_flat = xt_d.ap().rearrange("d b t -> d (b t)")
            nchunk = BPC * T // 512
            for ch in range(nchunk):
                xc = p1in.tile([128, 512], f32, tag="xc")
                nc.sync.dma_start(xc[:], xt_flat[:, ch * 512:(ch + 1) * 512])
                xtb = p1xt.tile([128, 512], bf16, tag="xtb")
                nc.scalar.activation(xtb[:], xc[:], AF.Copy)
                for g in range(4):
                    ps = p1ps.tile([128, 512], f32, tag="ps")
                    nc.tensor.matmul(
                        ps[:],
                        wb_sb[:, g * 128:(g + 1) * 128],
                        xtb[:],
                        start=True,
                        stop=True,
                    )
                    # dest view [128, 512] over (bb, t) of this chunk
                    dst = xwb[:, g, :, :].rearrange("p b t -> p (b t)")[
                        :, ch * 512:(ch + 1) * 512
                    ]
                    nc.scalar.activation(
                        dst, ps[:], AF.Identity, bias=bias_sb[:, g:g + 1]
                    )

        # ---------------- Phase 2: recurrence ----------------
        # Per-step access patterns are static; the only dynamic APs are three
        # block-granularity copies per For_i iteration.
        st = ctx.enter_context(tc.tile_pool(name="state", bufs=1))
        gp = ctx.enter_context(tc.tile_pool(name="gates", bufs=2))
        zp = ctx.enter_context(tc.tile_pool(name="zps", bufs=2, space="PSUM"))

        hT = st.tile([128, BPC], bf16, tag="hT")
        stage = st.tile([128, 4, BPC, UNROLL], bf16, tag="stage")
        hblk = st.tile([128, BPC, UNROLL], f32, tag="hblk")
        cblk = st.tile([128, BPC, UNROLL], f32, tag="cblk")
        nc.vector.memset(hT[:], 0.0)
        nc.vector.memset(cblk[:, :, UNROLL - 1:UNROLL], 0.0)

        def step(j):
            z = zp.tile([128, 4 * BPC], f32, tag="z")
            nc.tensor.matmul(
                z[:],
                idb_sb[:],
                stage[:, :, :, j:j + 1].rearrange("p a b c -> p (a b c)"),
                start=True,
                stop=False,
                skip_group_check=True,
            )
            for g in range(4):
                nc.tensor.matmul(
                    z[:, g * BPC:(g + 1) * BPC],
                    ub_sb[:, g * 128:(g + 1) * 128],
                    hT[:],
                    start=False,
                    stop=(g == 3),
                    skip_group_check=True,
                )
            s = gp.tile([128, 4 * BPC], bf16, tag="s")
            nc.scalar.activation(s[:], z[:], AF.Sigmoid)
            s_i = s[:, 0 * BPC:1 * BPC]
            s_f = s[:, 1 * BPC:2 * BPC]
            s_m = s[:, 2 * BPC:3 * BPC]
            s_o = s[:, 3 * BPC:4 * BPC]

            cprev = cblk[:, :, j - 1] if j > 0 else cblk[:, :, UNROLL - 1]
            cnew = cblk[:, :, j]
            q = gp.tile([128, BPC], f32, tag="q")
            u = gp.tile([128, BPC], f32, tag="u")
            t1 = gp.tile([128, BPC], f32, tag="t1")
            # q = (-2*m)*i ; u = i + q ; t1 = f*c_prev ; c = u + t1
            nc.vector.scalar_tensor_tensor(q[:], s_m, -2.0, s_i, OP.mult, OP.mult)
            nc.vector.tensor_add(u[:], s_i, q[:])
            nc.vector.tensor_mul(t1[:], s_f, cprev)
            nc.vector.tensor_add(cnew, u[:], t1[:])

            tcz = gp.tile([128, BPC], bf16, tag="tcz")
            nc.scalar.activation(tcz[:], cnew, AF.Tanh)
            nc.vector.tensor_mul(hT[:], s_o, tcz[:])
            nc.vector.tensor_mul(hblk[:, :, j], s_o, tcz[:])

        import concourse.bass as bass_

        if USE_FOR_I:
            with tc.For_i(0, T // UNROLL, 1) as it:
                tb = bass_.ds(it * UNROLL, UNROLL)
                nc.scalar.activation(stage[:], xwb[:, :, :, tb], AF.Copy)
                for j in range(UNROLL):
                    step(j)
                nc.vector.tensor_copy(hsb[:, :, tb], hblk[:])
                nc.vector.tensor_copy(csb[:, :, tb], cblk[:])
        else:
            for blk in range(T // UNROLL):
                tb = slice(blk * UNROLL, (blk + 1) * UNROLL)
                nc.scalar.activation(stage[:], xwb[:, :, :, tb], AF.Copy)
                for j in range(UNROLL):
                    step(j)
                nc.vector.tensor_copy(hsb[:, :, tb], hblk[:])
                nc.vector.tensor_copy(csb[:, :, tb], cblk[:])

        # ---------------- Phase 3: outputs ----------------
        for bb in range(BPC):
            for half in range(2):
                sl = slice(half * (T // 2), (half + 1) * (T // 2))
                nc.sync.dma_start(hs_d.ap()[:, bb, sl], hsb[:, bb, sl])
                nc.sync.dma_start(cs_d.ap()[:, bb, sl], csb[:, bb, sl])

    nc.compile()
    return nc


def _get_program():
    if "nc" not in _CACHED:
        _CACHED["nc"] = _build_program()
    return _CACHED["nc"]


def kernel(x, W, U, b_i, b_f, b_c, b_o, trace=False):
    from concourse.bass_utils import run_bass_kernel_spmd

    bf16 = ml_dtypes.bfloat16
    x = np.asarray(x, dtype=np.float32)
    W = np.asarray(W, dtype=np.float32).copy()
    U = np.asarray(U, dtype=np.float32).copy()
    bias = np.stack(
        [np.asarray(b_i), np.asarray(b_f), np.asarray(b_c), np.asarray(b_o)], 1
    ).astype(np.float32)

    # sigmoid-trick pre-scaling: gate g (cols 256:384) by -2
    W[:, 256:384] *= -2.0
    U[:, 256:384] *= -2.0
    bias[:, 2] *= -2.0
    Wb = W.astype(bf16)
    Ub = U.astype(bf16)
    idb = np.eye(128).astype(bf16)

    in_maps = []
    for k in range(NCORES):
        xk = x[k * BPC:(k + 1) * BPC]           # [BPC, T, D]
        xtk = np.ascontiguousarray(xk.transpose(2, 0, 1))  # [D, BPC, T]
        in_maps.append({
            "xt": xtk,
            "wb": Wb,
            "ub": Ub,
            "bias": bias,
            "idb": idb,
        })

    nc = _get_program()
    res = run_bass_kernel_spmd(nc, in_maps, list(range(NCORES)), trace=trace)

    hs = np.empty((B, T, H), np.float32)
    cs = np.empty((B, T, H), np.float32)
    for k in range(NCORES):
        hs_dev = res.results[k]["hs"]  # [H, BPC, T]
        cs_dev = res.results[k]["cs"]
        for bb in range(BPC):
            hs[k * BPC + bb] = hs_dev[:, bb, :].T
            cs[k * BPC + bb] = cs_dev[:, bb, :].T
    if trace:
        kernel.last_result = res
    return hs, cs
